# revision 21
# baseline (speedup 1.0000x reference)
"""Routed MoE classifier head for Trainium2 (8 NeuronCores, SPMD).

The reference computes all 8 experts densely and selects; here each sample is
routed to exactly one expert.  On the host we gather samples by expert
(expert e -> core e), pad to a common S, and pre-transpose x so the
contraction dim D lands on SBUF partitions.  Each core runs a dense 2-layer
MLP (768 -> relu 384 -> 8) over its expert's samples:

  layer 1:  h^T = relu(W1^T x^T + b1)   as matmul(psum, lhsT=W1 [128,128],
            rhs=xT [128,n]) accumulated over 6 d-blocks per h-block
  layer 2:  y^T = W2^T h^T + b2

Matmul operands use float32r (fp32 bits, 1 column/cycle streaming — 4x the
fp32 paired-pass rate — with ~11-mantissa-bit operand rounding); PSUM
accumulation stays fp32.  Output y^T [8, S] is scattered back on the host.
"""

import numpy as np

import concourse.bass as bass
import concourse.mybir as mybir
from concourse.tile import TileContext, add_dep_helper
from concourse.bass_utils import run_bass_kernel_spmd

P = 128
D = 768
H = 384
C = 8
E = 8
NCORES = 8
DBLK = D // P  # 6
HBLK = H // P  # 3
CHUNK = 512  # compute chunk (one PSUM bank of fp32)
XGRAN = 1024  # x DMA granularity (samples per load)
YGRAN = 4096  # y DMA granularity (samples per store)

MM_DTYPE = "f32r"

_program_cache = {}
last_results = None  # BassKernelResults of the most recent run (for test harness)


def _split_excess_waits(nc, max_waits=1):
    """The walrus build in this container only encodes one sem-wait per
    instruction; hoist extra waits onto NOPs inserted just before."""
    for blk in nc.main_func.blocks:
        insts = blk.instructions
        i = 0
        while i < len(insts):
            inst = insts[i]
            si = getattr(inst, "sync_info", None)
            if si is not None and si.on_wait and len(si.on_wait) > max_waits:
                waits = list(si.on_wait)
                extra, keep = waits[:-max_waits], waits[-max_waits:]
                nops = []
                for j in range(0, len(extra), max_waits):
                    nops.append(
                        mybir.InstNoOp(
                            name=f"{inst.name}-wsplit{j}",
                            engine=inst.engine,
                            bass_nofuse=True,
                            sync_info=mybir.SyncInfo(
                                on_wait=extra[j : j + max_waits], on_update=[]
                            ),
                        )
                    )
                inst.sync_info = mybir.SyncInfo(on_wait=keep, on_update=si.on_update)
                for k, nop in enumerate(nops):
                    nc.register_instruction(nop, overwrite=True)
                    insts.insert(i + k, nop)
                i += len(nops)
            i += 1
    return nc


def _spans(total, first, gran):
    """[(off, n), ...] covering `total`: one leading span of `first`, then
    `gran`-sized spans (last one smaller)."""
    spans = []
    off = 0
    n = min(first, total)
    while off < total:
        spans.append((off, n))
        off += n
        n = min(gran, total - off)
    return spans


def _build_program(S):
    f32 = mybir.dt.float32
    fmm = mybir.dt.float32r if MM_DTYPE == "f32r" else f32
    relu = mybir.ActivationFunctionType.Relu
    add = mybir.AluOpType.add

    nc = bass.Bass(enable_partition_id=False)
    xt = nc.dram_tensor("xt", [P, DBLK, S], fmm, kind="ExternalInput")
    # w1 (6*384 cols) and w2 (3*8 cols) packed on the same 128 partitions
    wt = nc.dram_tensor("wt", [P, DBLK * H + HBLK * C], fmm, kind="ExternalInput")
    # b1 (3 cols, per h-block) and b2 (1 col, rows 0..7) packed
    bt = nc.dram_tensor("bt", [P, HBLK + 1], f32, kind="ExternalInput")
    yt = nc.dram_tensor("yt", [C, S], f32, kind="ExternalOutput")

    x_spans = _spans(S, CHUNK, XGRAN)
    y_spans = _spans(S, YGRAN, YGRAN)

    with TileContext(nc) as tc:
        with (
            tc.tile_pool(name="const", bufs=1) as cpool,
            tc.tile_pool(name="xin", bufs=4) as xpool,
            tc.tile_pool(name="hbuf", bufs=3) as hpool,
            tc.tile_pool(name="yout", bufs=2) as ypool,
            tc.tile_pool(name="psum1", bufs=6, space="PSUM") as pp1,
            tc.tile_pool(name="psum2", bufs=2, space="PSUM") as pp2,
        ):
            w_t = cpool.tile([P, DBLK * H + HBLK * C], fmm)
            nc.sync.dma_start(w_t[:], wt[:])
            b_t = cpool.tile([P, HBLK + 1], f32)
            nc.sync.dma_start(b_t[:], bt[:])

            span_tiles = {}

            def load_x(span_idx):
                off, n = x_spans[span_idx]
                x_t = xpool.tile([P, DBLK, XGRAN], fmm, name="x_t")
                if span_idx == 0:
                    for db in range(DBLK):
                        nc.sync.dma_start(x_t[:, db, :n], xt[:, db, off : off + n])
                else:
                    nc.sync.dma_start(x_t[:, :, :n], xt[:, :, off : off + n])
                span_tiles[span_idx] = x_t

            y_tile = None  # current [C, YGRAN] output staging tile
            y_base = 0

            def emit_l2(pend):
                # layer 2 for an already-relu'd chunk: y^T = W2^T h^T + b2
                nonlocal y_tile, y_base
                h_t, off, n = pend
                ps2 = pp2.tile([C, CHUNK], f32, name="ps2")
                for hb in range(HBLK):
                    nc.tensor.matmul(
                        ps2[:, :n],
                        w_t[:, DBLK * H + hb * C : DBLK * H + (hb + 1) * C],
                        h_t[:, hb, :n],
                        start=(hb == 0),
                        stop=(hb == HBLK - 1),
                    )
                if y_tile is None:
                    y_tile = ypool.tile([C, YGRAN], f32, name="y_t")
                    y_base = off
                lo = off - y_base
                nc.vector.tensor_scalar(
                    y_tile[:, lo : lo + n],
                    ps2[:, :n],
                    scalar1=b_t[:C, HBLK : HBLK + 1],
                    scalar2=None,
                    op0=add,
                )
                if lo + n >= YGRAN or off + n >= S:
                    nc.sync.dma_start(yt[:, y_base : y_base + lo + n], y_tile[:, : lo + n])
                    y_tile = None

            # Software pipeline: emit layer-2 of chunk k-1 between layer-1 of
            # chunk k and k+1 so the PE never waits on the ACT-relu epilogue.
            load_x(0)
            pending = None
            for si, (soff, sn) in enumerate(x_spans):
                x_t = span_tiles.pop(si)
                for o in range(0, sn, CHUNK):
                    n = min(CHUNK, sn - o)
                    h_t = hpool.tile([P, HBLK, CHUNK], fmm, name="h_t")
                    pss = [
                        pp1.tile([P, CHUNK], f32, name="ps") for _ in range(HBLK)
                    ]
                    for db in range(DBLK):
                        for hb in range(HBLK):
                            nc.tensor.matmul(
                                pss[hb][:, :n],
                                w_t[:, db * H + hb * P : db * H + (hb + 1) * P],
                                x_t[:, db, o : o + n],
                                start=(db == 0),
                                stop=(db == DBLK - 1),
                            )
                    for hb in range(HBLK):
                        nc.scalar.activation(
                            h_t[:, hb, :n], pss[hb][:, :n], relu,
                            bias=b_t[:, hb : hb + 1],
                        )
                    if o == 0 and si + 1 < len(x_spans):
                        load_x(si + 1)
                    if pending is not None:
                        emit_l2(pending)
                    pending = (h_t, soff + o, n)
            emit_l2(pending)

    return _split_excess_waits(nc)


def kernel(x, W1, b1, W2, b2, question_types):
    global last_results
    x = np.ascontiguousarray(np.asarray(x, dtype=np.float32))
    W1 = np.asarray(W1, dtype=np.float32)
    b1 = np.asarray(b1, dtype=np.float32)
    W2 = np.asarray(W2, dtype=np.float32)
    b2 = np.asarray(b2, dtype=np.float32)
    qt = np.asarray(question_types)
    N = x.shape[0]

    idx = [np.nonzero(qt == e)[0] for e in range(E)]
    counts = [len(i) for i in idx]
    S = max(int(np.ceil(max(counts) / 16) * 16), 2 * CHUNK)

    nc = _program_cache.get(S)
    if nc is None:
        nc = _build_program(S)
        _program_cache[S] = nc

    in_maps = []
    for e in range(E):
        cnt = counts[e]
        xp = np.zeros((S, D), np.float32)
        xp[:cnt] = x[idx[e]]
        xt = np.ascontiguousarray(xp.T.reshape(DBLK, P, S).transpose(1, 0, 2))
        w1t = W1[e].reshape(DBLK, P, H).transpose(1, 0, 2).reshape(P, DBLK * H)
        w2t = W2[e].reshape(HBLK, P, C).transpose(1, 0, 2).reshape(P, HBLK * C)
        wt = np.ascontiguousarray(np.concatenate([w1t, w2t], axis=1))
        bt = np.zeros((P, HBLK + 1), np.float32)
        bt[:, :HBLK] = b1[e].reshape(HBLK, P).T
        bt[:C, HBLK] = b2[e]
        in_maps.append({"xt": xt, "wt": wt, "bt": bt})

    r = run_bass_kernel_spmd(nc, in_maps, list(range(NCORES)))
    last_results = r

    out = np.zeros((N, C), np.float32)
    for e in range(E):
        out[idx[e]] = r.results[e]["yt"][:, : counts[e]].T
    return out
